# revision 22
# baseline (speedup 1.0000x reference)
"""Trainium2 Bass kernel for nn_CNNFromScratch (dense 1-D CNN + MLP head).

Strategy
--------
Pure data parallelism: the batch axis (8192) is split across 8 NeuronCores
(1024 samples each); conv kernels and MLP weights are replicated.

Per core, everything is expressed as TensorE matmuls with the contraction
(input channels x taps) on the partition axis:

  - x is host-packed per (tile, c-chunk) so every x DMA is one fully
    contiguous run per partition. All input DMAs are issued back-to-back
    on the sync engine's HWDGE ring (FIFO -> sequential completions, no
    per-DMA dep-chain latency), ordered earliest-needed-first; bulk
    weights ride the same ring as a single packed tensor.
  - conv_k == sum over taps of  W_tap^T @ x[:, :, w+tap]  accumulated in
    PSUM. Activations stay on-chip (SBUF, bf16) between layers.
  - Conv loops are weight-stationary: one LDWEIGHTS per weight block, then
    a run of matmuls with ldweights=False that stream different rhs/PSUM
    slices through the already-loaded array (a per-MM weight swap costs
    ~43ns on top of the N-cycle stream; elision runs at stream rate).
    All PE matmuls carry explicit same-engine ordering deps so the
    scheduler cannot move a follower away from its weight load.
  - conv1 packs output positions in (even, odd) pairs on PSUM partitions
    0-63 / 64-127; the two M=64 matmuls of a pair target different PE
    column groups and execute concurrently, sharing one LDWEIGHTS of a
    host-duplicated 128-column block.
  - For bt=256 tiles, two positions' accumulators share one PSUM bank
    (free-dim split) and drain with a single wide Relu — halves the
    cross-engine sync count. h2 is parity-major so conv2's paired drains
    stay contiguous.
  - Emission order: conv1(t) | mlp(t-1) | conv2+conv3+pool(t), so each
    tile's serial pool->mlp drain chain hides inside the next tile's
    conv window.

Matmul inputs are bf16 (1 cycle/row on PE), accumulation is fp32 in PSUM.
"""

import sys

sys.path.insert(0, "/opt/trn_rl_repo")

import numpy as np
import ml_dtypes

N_CORES = 8
B, E, W = 8192, 512, 20
BC = B // N_CORES  # samples per core
# Small first tile shortens the DMA-gated pipeline fill; small last tile
# shortens the serial mlp/drain tail.
TILES = [256, 512, 256]
assert sum(TILES) == BC
MAXBT = max(TILES)

# Mark weight-stationary followers with ldweights=False. Measured to be a
# no-op in this toolchain (LDWEIGHTS still emitted 1:1, numerics identical
# either way); the cadence win comes from consecutive same-weight loads not
# paying the array swap stall. Kept off so behavior never depends on it.
ELIDE_LDW = False

BF16 = ml_dtypes.bfloat16

# h2 column order is parity-major: even output positions first (7), then
# odd (7). Keeps conv2's paired PSUM drains contiguous in h2.
C2 = [w // 2 + (0 if w % 2 == 0 else 7) for w in range(14)]

_compiled = {}


def _pairs(lo, hi, bp):
    """Split positions [lo, hi) into runs of length bp (last may be short)."""
    out = []
    i = lo
    while i < hi:
        out.append(list(range(i, min(i + bp, hi))))
        i += bp
    return out


def _build():
    import concourse.bass as bass
    from concourse import bacc, mybir
    import concourse.tile as tile

    dt = mybir.dt
    AF = mybir.ActivationFunctionType

    nc = bacc.Bacc(
        "TRN2",
        target_bir_lowering=False,
        debug=False,
        enable_asserts=False,
        num_devices=N_CORES,
    )

    NT = len(TILES)
    # Flat per-tile packing: tile ti occupies columns [W*off, W*(off+bt)).
    x_d = nc.dram_tensor("x", (E, W * BC), dt.bfloat16, kind="ExternalInput").ap()
    tile_off = [W * sum(TILES[:t]) for t in range(NT)]
    # conv1 weights: 4 c-chunks side by side -> one DMA.
    w1_d = nc.dram_tensor("w1", (128, 4 * 384), dt.bfloat16, kind="ExternalInput").ap()
    # All post-conv1 weights host-packed: [w2 | w3 | m1 | m2 | m3].
    WB_COLS = 6 * 128 + 7 * 256 + 8 * 256 + 2 * 128 + 1
    wb_d = nc.dram_tensor("wb", (128, WB_COLS), dt.bfloat16, kind="ExternalInput").ap()
    y_d = nc.dram_tensor("y", (1, BC), dt.float32, kind="ExternalOutput").ap()

    with tile.TileContext(nc) as tc:
        with (
            tc.tile_pool(name="sb", bufs=1) as sb,
            tc.tile_pool(name="ps", bufs=8, space="PSUM") as ps,
        ):
            # ---- global PE ordering chain ----
            pe_chain = [None]

            def mm(*args, **kw):
                inst = nc.tensor.matmul(*args, **kw)
                if pe_chain[0] is not None:
                    tile.add_dep_helper(
                        inst.ins, pe_chain[0], reason="PE program order"
                    )
                pe_chain[0] = inst.ins
                return inst

            def mm_f(*args, **kw):
                inst = mm(*args, **kw)
                if ELIDE_LDW:
                    inst.ins.ldweights = False
                return inst

            # ---- PE warmup: engine-generated data (no DMA dependency) so
            # the HAM clock ramp starts as soon as the start barrier clears.
            warm_in = sb.tile([128, 192], dt.bfloat16, tag="warm_in")
            nc.gpsimd.memset(warm_in[:], 0.0)
            warm_ps = ps.tile([128, 512], dt.float32, tag="ps", name="warm_ps")
            for _ in range(18):
                mm(
                    warm_ps[0:64, 0:192],
                    warm_in[:, 0:64],
                    warm_in[:, :],
                    start=True,
                    stop=True,
                )

            # ---- input DMAs: one FIFO ring (sync engine), issue order =
            # completion order, earliest-needed-first ----
            w1_all = sb.tile([128, 4 * 384], dt.bfloat16, tag="w1")
            nc.sync.dma_start(w1_all[:], w1_d[:, :])
            w1_sb = [w1_all[:, q * 384 : (q + 1) * 384] for q in range(4)]

            def x_slot(ti, q):
                return sb.tile(
                    [128, MAXBT * 20],
                    dt.bfloat16,
                    tag="x",
                    bufs=6,
                    name=f"x_{ti}_{q}",
                )

            # tile-0 x in two w-halves per chunk: the first half (w0..9) is
            # all conv1's u-block A needs, so the PE starts sooner.
            bt0 = TILES[0]
            x0_slots = [x_slot(0, q) for q in range(4)]
            for q in range(4):
                nc.sync.dma_start(
                    x0_slots[q][:, : bt0 * 10],
                    x_d[q * 128 : (q + 1) * 128, 0 : 10 * bt0],
                )
            for q in range(4):
                nc.sync.dma_start(
                    x0_slots[q][:, bt0 * 10 : bt0 * 20],
                    x_d[q * 128 : (q + 1) * 128, 10 * bt0 : 20 * bt0],
                )
            x_tiles = [[t[:, : bt0 * 20] for t in x0_slots]]

            # ACT Relu table preload off the critical path.
            warm_act = sb.tile([1, 1], dt.float32, tag="warm_act")
            nc.scalar.activation(warm_act[:], w1_all[0:1, 0:1], AF.Relu)

            # Bulk weights (single DMA; first needed when conv2 of tile 0
            # starts).
            wb_sb = sb.tile([128, WB_COLS], dt.bfloat16, tag="wb")
            nc.sync.dma_start(wb_sb[:], wb_d[:, :])
            c = 0
            w2_sb = wb_sb[:, c : c + 6 * 128]; c += 6 * 128
            w3_sb = wb_sb[:, c : c + 7 * 256]; c += 7 * 256
            m1_sb = []
            for wp in range(4):
                row = []
                for q in range(2):
                    row.append(wb_sb[:, c : c + 256]); c += 256
                m1_sb.append(row)
            m2_sb = []
            for q in range(2):
                m2_sb.append(wb_sb[:, c : c + 128]); c += 128
            m3_sb = wb_sb[:, c : c + 1]; c += 1
            assert c == WB_COLS

            # Remaining tiles' x.
            for ti in range(1, NT):
                bt = TILES[ti]
                off = tile_off[ti]
                slots = [x_slot(ti, q) for q in range(4)]
                for q in range(4):
                    nc.sync.dma_start(
                        slots[q][:, : bt * 20],
                        x_d[q * 128 : (q + 1) * 128, off : off + 20 * bt],
                    )
                x_tiles.append([t[:, : bt * 20] for t in slots])

            # ---- per-batch-tile pipeline ----
            tile_offs = []
            _boff = 0
            for bt in TILES:
                tile_offs.append(_boff)
                _boff += bt

            def emit_conv1(ti):
                bt = TILES[ti]
                bp = 1  # DIAG: disable free-dim PSUM pairing
                x_sb = x_tiles[ti]

                # conv1: (B,512,20) -> relu -> (B,64,18)
                # (even, odd) position pairs on PSUM partition halves; both
                # halves of a pair share one LDWEIGHTS. Chunk-outer,
                # weight-stationary inside (q, k).
                h1 = sb.tile([128, 9 * MAXBT], dt.bfloat16, tag="h1")
                for u0, u1 in ((0, 4), (4, 9)):
                    banks = _pairs(u0, u1, bp)
                    pt = {}
                    bank_tiles = []
                    for bank in banks:
                        t = ps.tile(
                            [128, len(bank) * bt], dt.float32,
                            tag="ps", name=f"p1_{ti}_{bank[0]}",
                        )
                        bank_tiles.append((bank, t))
                        for i, u in enumerate(bank):
                            pt[u] = t[:, i * bt : (i + 1) * bt]
                    for q in range(4):
                        for k in range(3):
                            lead = True
                            for u in range(u0, u1):
                                for half in range(2):
                                    f = mm if lead else mm_f
                                    f(
                                        pt[u][half * 64 : half * 64 + 64, :],
                                        w1_sb[q][:, k * 128 + half * 64 : k * 128 + half * 64 + 64],
                                        x_sb[q][:, (2 * u + half + k) * bt : (2 * u + half + k + 1) * bt],
                                        start=(q == 0 and k == 0),
                                        stop=(q == 3 and k == 2),
                                        skip_group_check=True,
                                    )
                                    lead = False
                    for bank, t in bank_tiles:
                        nc.scalar.activation(
                            h1[:, bank[0] * bt : (bank[-1] + 1) * bt],
                            t[:, : len(bank) * bt],
                            AF.Relu,
                        )
                return h1

            def emit_mid(ti, h1):
                bt = TILES[ti]
                bp = 1

                # conv2: -> relu -> (B,128,14), h2 parity-major (C2 map).
                # Weight-stationary per parity group.
                h2 = sb.tile([128, 14 * MAXBT], dt.bfloat16, tag="h2")
                for grp in ((0, 2, 4, 6), (1, 3, 5, 7), (8, 10, 12), (9, 11, 13)):
                    banks = _pairs(0, len(grp), bp)
                    pt = {}
                    bank_tiles = []
                    for bank in banks:
                        t = ps.tile(
                            [128, len(bank) * bt], dt.float32,
                            tag="ps", name=f"p2_{ti}_{grp[bank[0]]}",
                        )
                        bank_tiles.append((bank, t))
                        for i, gi in enumerate(bank):
                            pt[grp[gi]] = t[:, i * bt : (i + 1) * bt]
                    blk0 = 0 if grp[0] % 2 == 0 else 3
                    for j in range(3):
                        lead = True
                        for w in grp:
                            f = mm if lead else mm_f
                            f(
                                pt[w][:],
                                w2_sb[:, (blk0 + j) * 128 : (blk0 + j + 1) * 128],
                                h1[:, (w // 2 + j) * bt : (w // 2 + j + 1) * bt],
                                start=(j == 0),
                                stop=(j == 2),
                                skip_group_check=True,
                            )
                            lead = False
                    for bank, t in bank_tiles:
                        c0 = C2[grp[bank[0]]]
                        nc.vector.tensor_relu(
                            h2[:, c0 * bt : (c0 + len(bank)) * bt],
                            t[:, : len(bank) * bt],
                        )

                # conv3: -> relu -> (B,256,8) as two 128-channel tiles,
                # weight-stationary over 4-position blocks.
                h3 = [
                    sb.tile([128, 8 * MAXBT], dt.bfloat16, tag=f"h3_{m}", name=f"h3_{m}")
                    for m in range(2)
                ]
                for m in range(2):
                    for w0 in (0, 4):
                        banks = _pairs(w0, w0 + 4, bp)
                        pt = {}
                        bank_tiles = []
                        for bank in banks:
                            t = ps.tile(
                                [128, len(bank) * bt], dt.float32,
                                tag="ps", name=f"p3_{ti}_{m}_{bank[0]}",
                            )
                            bank_tiles.append((bank, t))
                            for i, w in enumerate(bank):
                                pt[w] = t[:, i * bt : (i + 1) * bt]
                        for k in range(7):
                            lead = True
                            for w in range(w0, w0 + 4):
                                f = mm if lead else mm_f
                                f(
                                    pt[w][:],
                                    w3_sb[:, k * 256 + m * 128 : k * 256 + (m + 1) * 128],
                                    h2[:, C2[w + k] * bt : (C2[w + k] + 1) * bt],
                                    start=(k == 0),
                                    stop=(k == 6),
                                    skip_group_check=True,
                                )
                                lead = False
                        for bank, t in bank_tiles:
                            nc.vector.tensor_relu(
                                h3[m][:, bank[0] * bt : (bank[-1] + 1) * bt],
                                t[:, : len(bank) * bt],
                            )

                # maxpool k=2 s=2: (B,256,8) -> (B,256,4)
                pooled = [
                    sb.tile([128, 4 * MAXBT], dt.bfloat16, tag=f"pool_{m}", name=f"pool_{m}")
                    for m in range(2)
                ]
                for m in range(2):
                    for p in range(4):
                        nc.vector.tensor_max(
                            pooled[m][:, p * bt : (p + 1) * bt],
                            h3[m][:, (2 * p) * bt : (2 * p + 1) * bt],
                            h3[m][:, (2 * p + 1) * bt : (2 * p + 2) * bt],
                        )
                return pooled

            def emit_mlp(ti, pooled):
                bt = TILES[ti]
                boff = tile_offs[ti]

                # mlp1: (B,1024)->(B,256), f = c*4 + wp. For bt=256 both
                # j-halves share one bank and drain with one Relu.
                g1 = sb.tile([128, 2 * MAXBT], dt.bfloat16, tag="g1")
                if False:
                    pmj = ps.tile([128, 2 * bt], dt.float32, tag="ps", name=f"pm1_{ti}")
                    pms = [pmj[:, 0:bt], pmj[:, bt : 2 * bt]]
                else:
                    pms = [
                        ps.tile([128, bt], dt.float32, tag="ps", name=f"pm1_{ti}_{j}")
                        for j in range(2)
                    ]
                for j in range(2):
                    for wp in range(4):
                        for q in range(2):
                            mm(
                                pms[j][:],
                                m1_sb[wp][q][:, j * 128 : (j + 1) * 128],
                                pooled[q][:, wp * bt : (wp + 1) * bt],
                                start=(wp == 0 and q == 0),
                                stop=(wp == 3 and q == 1),
                                skip_group_check=True,
                            )
                if False:
                    nc.vector.tensor_relu(g1[:, 0 : 2 * bt], pmj[:])
                else:
                    for j in range(2):
                        nc.vector.tensor_relu(
                            g1[:, j * bt : (j + 1) * bt], pms[j][:]
                        )

                # mlp2: (B,256)->(B,128)
                g2 = sb.tile([128, MAXBT], dt.bfloat16, tag="g2")
                pm = ps.tile([128, bt], dt.float32, tag="ps", name=f"pm2_{ti}")
                for q in range(2):
                    mm(
                        pm[:], m2_sb[q][:], g1[:, q * bt : (q + 1) * bt],
                        start=(q == 0), stop=(q == 1),
                    )
                nc.vector.tensor_relu(g2[:, :bt], pm[:])

                # mlp3: (B,128)->(B,1)
                pm = ps.tile([1, bt], dt.float32, tag="ps", name=f"pm3_{ti}")
                mm(pm[:], m3_sb[:], g2[:, :bt], start=True, stop=True)
                y_sb = sb.tile([1, MAXBT], dt.float32, tag="y_sb", bufs=2)
                nc.vector.tensor_copy(y_sb[:, :bt], pm[:])
                nc.scalar.dma_start(y_d[:, boff : boff + bt], y_sb[:, :bt])

            pooled_prev = None
            for ti in range(NT):
                h1 = emit_conv1(ti)
                if pooled_prev is not None:
                    emit_mlp(ti - 1, pooled_prev)
                pooled_prev = emit_mid(ti, h1)
            emit_mlp(NT - 1, pooled_prev)

    nc.compile()
    return nc


def _prep_inputs(x, kernel_1, kernel_2, kernel_3, mlp_weight_1, mlp_weight_2, mlp_weight_3):
    """Host-side sharding + layout prep. Returns in_maps (one dict per core)."""
    # conv1 taps duplicated across both 64-column halves so one 128-col
    # LDWEIGHTS serves the (even, odd) pair of M=64 matmuls; the four
    # c-chunks packed side by side for a single DMA.
    k1t = kernel_1.transpose(1, 2, 0).astype(np.float32)  # (512, 3, 64)
    w1 = np.concatenate([k1t, k1t], axis=2).reshape(512, 3 * 128)
    w1 = np.ascontiguousarray(
        w1.reshape(4, 128, 384).transpose(1, 0, 2).reshape(128, 4 * 384)
    ).astype(BF16)
    # conv2 tap-pair blocks for the parity-split h1 layout: column block j is
    # a (128, 128) lhsT whose rows 0-63 multiply h1's even half and rows
    # 64-127 the odd half. Blocks 0-2 serve even output positions
    # ([k0;k1] [k2;k3] [k4;0]), blocks 3-5 odd ones ([0;k0] [k1;k2] [k3;k4]).
    k2t = kernel_2.transpose(1, 2, 0).astype(np.float32)  # (64, 5, 128)
    z = np.zeros((64, 128), np.float32)
    blocks = [
        np.concatenate([k2t[:, 0], k2t[:, 1]], axis=0),
        np.concatenate([k2t[:, 2], k2t[:, 3]], axis=0),
        np.concatenate([k2t[:, 4], z], axis=0),
        np.concatenate([z, k2t[:, 0]], axis=0),
        np.concatenate([k2t[:, 1], k2t[:, 2]], axis=0),
        np.concatenate([k2t[:, 3], k2t[:, 4]], axis=0),
    ]
    w2 = np.concatenate(blocks, axis=1)  # (128, 768)
    w3 = kernel_3.transpose(1, 2, 0).reshape(128, 7 * 256)
    # W1 row f = c*4 + wp  ->  m1 row = wp*256 + c; packed as 8 side-by-side
    # (128, 256) blocks in (wp, q) order.
    m1 = (
        mlp_weight_1.reshape(256, 4, 256)
        .transpose(1, 0, 2)
        .reshape(4, 2, 128, 256)
        .transpose(2, 0, 1, 3)
        .reshape(128, 8 * 256)
    )
    m2 = np.concatenate([mlp_weight_2[0:128], mlp_weight_2[128:256]], axis=1)
    m3 = mlp_weight_3  # (128, 1)
    wb = np.ascontiguousarray(
        np.concatenate([w2, w3, m1, m2, m3], axis=1)
    ).astype(BF16)

    xb = x.astype(BF16)
    in_maps = []
    for c in range(N_CORES):
        # (E, W*BC), tile ti packed at columns [W*off, W*(off+bt)): each
        # (c-chunk, tile) DMA is one contiguous 20*bt*2-byte run/partition.
        xc = np.empty((E, W * BC), dtype=BF16)
        boff = 0
        for bt in TILES:
            blk = xb[c * BC + boff : c * BC + boff + bt]  # (bt, E, W)
            xc[:, W * boff : W * (boff + bt)] = blk.transpose(1, 2, 0).reshape(E, W * bt)
            boff += bt
        in_maps.append({"x": xc, "w1": w1, "wb": wb})
    return in_maps


def run(inputs, trace=False, **kw):
    """Compile (cached), run on 8 cores, return (y_full, BassKernelResults)."""
    from concourse import bass_utils

    if "nc" not in _compiled:
        _compiled["nc"] = _build()
    nc = _compiled["nc"]
    in_maps = _prep_inputs(**inputs)
    res = bass_utils.run_bass_kernel_spmd(
        nc, in_maps, core_ids=list(range(N_CORES)), trace=trace, **kw
    )
    y = np.concatenate(
        [res.results[c]["y"].reshape(BC, 1) for c in range(N_CORES)], axis=0
    )
    return y.astype(np.float32), res


def kernel(**inputs):
    inputs = {k: np.asarray(v) for k, v in inputs.items()}
    y, _ = run(inputs)
    return y


if __name__ == "__main__":
    rng = np.random.default_rng(0)
    inputs = {
        "x": rng.standard_normal((B, E, W), dtype=np.float32),
        "kernel_1": rng.standard_normal((64, 512, 3), dtype=np.float32),
        "kernel_2": rng.standard_normal((128, 64, 5), dtype=np.float32),
        "kernel_3": rng.standard_normal((256, 128, 7), dtype=np.float32),
        "mlp_weight_1": rng.standard_normal((1024, 256), dtype=np.float32),
        "mlp_weight_2": rng.standard_normal((256, 128), dtype=np.float32),
        "mlp_weight_3": rng.standard_normal((128, 1), dtype=np.float32),
    }
    y = kernel(**inputs)
    print("out", y.shape, y.dtype, y[:4, 0])


# revision 23
# speedup vs baseline: 1.0164x; 1.0164x over previous
"""Trainium2 Bass kernel for nn_CNNFromScratch (dense 1-D CNN + MLP head).

Strategy
--------
Pure data parallelism: the batch axis (8192) is split across 8 NeuronCores
(1024 samples each); conv kernels and MLP weights are replicated.

Per core, everything is expressed as TensorE matmuls with the contraction
(input channels x taps) on the partition axis:

  - x is host-packed per (tile, c-chunk) so every x DMA is one fully
    contiguous run per partition. All input DMAs are issued back-to-back
    on the sync engine's HWDGE ring (FIFO -> sequential completions, no
    per-DMA dep-chain latency), ordered earliest-needed-first; bulk
    weights ride the same ring as a single packed tensor.
  - conv_k == sum over taps of  W_tap^T @ x[:, :, w+tap]  accumulated in
    PSUM. Activations stay on-chip (SBUF, bf16) between layers.
  - Conv loops are weight-stationary: one LDWEIGHTS per weight block, then
    a run of matmuls with ldweights=False that stream different rhs/PSUM
    slices through the already-loaded array (a per-MM weight swap costs
    ~43ns on top of the N-cycle stream; elision runs at stream rate).
    All PE matmuls carry explicit same-engine ordering deps so the
    scheduler cannot move a follower away from its weight load.
  - conv1 packs output positions in (even, odd) pairs on PSUM partitions
    0-63 / 64-127; the two M=64 matmuls of a pair target different PE
    column groups and execute concurrently, sharing one LDWEIGHTS of a
    host-duplicated 128-column block.
  - For bt=256 tiles, two positions' accumulators share one PSUM bank
    (free-dim split) and drain with a single wide Relu — halves the
    cross-engine sync count. h2 is parity-major so conv2's paired drains
    stay contiguous.
  - Emission order: conv1(t) | mlp(t-1) | conv2+conv3+pool(t), so each
    tile's serial pool->mlp drain chain hides inside the next tile's
    conv window.

Matmul inputs are bf16 (1 cycle/row on PE), accumulation is fp32 in PSUM.
"""

import sys

sys.path.insert(0, "/opt/trn_rl_repo")

import numpy as np
import ml_dtypes

N_CORES = 8
B, E, W = 8192, 512, 20
BC = B // N_CORES  # samples per core
# Small leading tiles shorten the DMA-gated pipeline fill; the 512 tail
# tile keeps per-MM overhead low once the pipeline is full (a 512 middle
# tile was tried and stalls ~2.5us waiting for its x).
TILES = [256, 256, 512]
assert sum(TILES) == BC
MAXBT = max(TILES)

# Mark weight-stationary followers with ldweights=False. Measured to be a
# no-op in this toolchain (LDWEIGHTS still emitted 1:1, numerics identical
# either way); the cadence win comes from consecutive same-weight loads not
# paying the array swap stall. Kept off so behavior never depends on it.
ELIDE_LDW = False

BF16 = ml_dtypes.bfloat16

# h2 column order is parity-major: even output positions first (7), then
# odd (7). Keeps conv2's paired PSUM drains contiguous in h2.
C2 = [w // 2 + (0 if w % 2 == 0 else 7) for w in range(14)]

_compiled = {}


def _pairs(lo, hi, bp):
    """Split positions [lo, hi) into runs of length bp (last may be short)."""
    out = []
    i = lo
    while i < hi:
        out.append(list(range(i, min(i + bp, hi))))
        i += bp
    return out


def _build():
    import concourse.bass as bass
    from concourse import bacc, mybir
    import concourse.tile as tile

    dt = mybir.dt
    AF = mybir.ActivationFunctionType

    nc = bacc.Bacc(
        "TRN2",
        target_bir_lowering=False,
        debug=False,
        enable_asserts=False,
        num_devices=N_CORES,
    )

    NT = len(TILES)
    # Flat per-tile packing: tile ti occupies columns [W*off, W*(off+bt)).
    x_d = nc.dram_tensor("x", (E, W * BC), dt.bfloat16, kind="ExternalInput").ap()
    tile_off = [W * sum(TILES[:t]) for t in range(NT)]
    # conv1 weights: 4 c-chunks side by side -> one DMA.
    w1_d = nc.dram_tensor("w1", (128, 4 * 384), dt.bfloat16, kind="ExternalInput").ap()
    # All post-conv1 weights host-packed: [w2 | w3 | m1 | m2 | m3].
    WB_COLS = 6 * 128 + 7 * 256 + 8 * 256 + 2 * 128 + 1
    wb_d = nc.dram_tensor("wb", (128, WB_COLS), dt.bfloat16, kind="ExternalInput").ap()
    y_d = nc.dram_tensor("y", (1, BC), dt.float32, kind="ExternalOutput").ap()

    with tile.TileContext(nc) as tc:
        with (
            tc.tile_pool(name="sb", bufs=1) as sb,
            tc.tile_pool(name="ps", bufs=8, space="PSUM") as ps,
        ):
            # ---- global PE ordering chain ----
            pe_chain = [None]

            def mm(*args, **kw):
                inst = nc.tensor.matmul(*args, **kw)
                if pe_chain[0] is not None:
                    tile.add_dep_helper(
                        inst.ins, pe_chain[0], reason="PE program order"
                    )
                pe_chain[0] = inst.ins
                return inst

            def mm_f(*args, **kw):
                inst = mm(*args, **kw)
                if ELIDE_LDW:
                    inst.ins.ldweights = False
                return inst

            # ---- PE warmup: engine-generated data (no DMA dependency) so
            # the HAM clock ramp starts as soon as the start barrier clears.
            warm_in = sb.tile([128, 192], dt.bfloat16, tag="warm_in")
            nc.gpsimd.memset(warm_in[:], 0.0)
            warm_ps = ps.tile([128, 512], dt.float32, tag="ps", name="warm_ps")
            for _ in range(18):
                mm(
                    warm_ps[0:64, 0:192],
                    warm_in[:, 0:64],
                    warm_in[:, :],
                    start=True,
                    stop=True,
                )

            # ---- input DMAs: one FIFO ring (sync engine), issue order =
            # completion order, earliest-needed-first ----
            w1_all = sb.tile([128, 4 * 384], dt.bfloat16, tag="w1")
            nc.sync.dma_start(w1_all[:], w1_d[:, :])
            w1_sb = [w1_all[:, q * 384 : (q + 1) * 384] for q in range(4)]

            def x_slot(ti, q):
                return sb.tile(
                    [128, MAXBT * 20],
                    dt.bfloat16,
                    tag="x",
                    bufs=6,
                    name=f"x_{ti}_{q}",
                )

            # tile-0 x in two w-halves per chunk: the first half (w0..9) is
            # all conv1's u-block A needs, so the PE starts sooner.
            bt0 = TILES[0]
            x0_slots = [x_slot(0, q) for q in range(4)]
            for q in range(4):
                nc.sync.dma_start(
                    x0_slots[q][:, : bt0 * 10],
                    x_d[q * 128 : (q + 1) * 128, 0 : 10 * bt0],
                )
            for q in range(4):
                nc.sync.dma_start(
                    x0_slots[q][:, bt0 * 10 : bt0 * 20],
                    x_d[q * 128 : (q + 1) * 128, 10 * bt0 : 20 * bt0],
                )
            x_tiles = [[t[:, : bt0 * 20] for t in x0_slots]]

            # ACT Relu table preload off the critical path.
            warm_act = sb.tile([1, 1], dt.float32, tag="warm_act")
            nc.scalar.activation(warm_act[:], w1_all[0:1, 0:1], AF.Relu)

            # Bulk weights (single DMA; first needed when conv2 of tile 0
            # starts).
            wb_sb = sb.tile([128, WB_COLS], dt.bfloat16, tag="wb")
            nc.sync.dma_start(wb_sb[:], wb_d[:, :])
            c = 0
            w2_sb = wb_sb[:, c : c + 6 * 128]; c += 6 * 128
            w3_sb = wb_sb[:, c : c + 7 * 256]; c += 7 * 256
            m1_sb = []
            for wp in range(4):
                row = []
                for q in range(2):
                    row.append(wb_sb[:, c : c + 256]); c += 256
                m1_sb.append(row)
            m2_sb = []
            for q in range(2):
                m2_sb.append(wb_sb[:, c : c + 128]); c += 128
            m3_sb = wb_sb[:, c : c + 1]; c += 1
            assert c == WB_COLS

            # Remaining tiles' x.
            for ti in range(1, NT):
                bt = TILES[ti]
                off = tile_off[ti]
                slots = [x_slot(ti, q) for q in range(4)]
                for q in range(4):
                    nc.sync.dma_start(
                        slots[q][:, : bt * 20],
                        x_d[q * 128 : (q + 1) * 128, off : off + 20 * bt],
                    )
                x_tiles.append([t[:, : bt * 20] for t in slots])

            # ---- per-batch-tile pipeline ----
            tile_offs = []
            _boff = 0
            for bt in TILES:
                tile_offs.append(_boff)
                _boff += bt

            def emit_conv1(ti):
                bt = TILES[ti]
                bp = 1  # DIAG: disable free-dim PSUM pairing
                x_sb = x_tiles[ti]

                # conv1: (B,512,20) -> relu -> (B,64,18)
                # (even, odd) position pairs on PSUM partition halves; both
                # halves of a pair share one LDWEIGHTS. Chunk-outer,
                # weight-stationary inside (q, k).
                h1 = sb.tile([128, 9 * MAXBT], dt.bfloat16, tag="h1")
                for u0, u1 in ((0, 4), (4, 9)):
                    banks = _pairs(u0, u1, bp)
                    pt = {}
                    bank_tiles = []
                    for bank in banks:
                        t = ps.tile(
                            [128, len(bank) * bt], dt.float32,
                            tag="ps", name=f"p1_{ti}_{bank[0]}",
                        )
                        bank_tiles.append((bank, t))
                        for i, u in enumerate(bank):
                            pt[u] = t[:, i * bt : (i + 1) * bt]
                    for q in range(4):
                        for k in range(3):
                            lead = True
                            for u in range(u0, u1):
                                for half in range(2):
                                    f = mm if lead else mm_f
                                    f(
                                        pt[u][half * 64 : half * 64 + 64, :],
                                        w1_sb[q][:, k * 128 + half * 64 : k * 128 + half * 64 + 64],
                                        x_sb[q][:, (2 * u + half + k) * bt : (2 * u + half + k + 1) * bt],
                                        start=(q == 0 and k == 0),
                                        stop=(q == 3 and k == 2),
                                        skip_group_check=True,
                                    )
                                    lead = False
                    for bank, t in bank_tiles:
                        nc.scalar.activation(
                            h1[:, bank[0] * bt : (bank[-1] + 1) * bt],
                            t[:, : len(bank) * bt],
                            AF.Relu,
                        )
                return h1

            def emit_mid(ti, h1):
                bt = TILES[ti]
                bp = 1

                # conv2: -> relu -> (B,128,14), h2 parity-major (C2 map).
                # Weight-stationary per parity group.
                h2 = sb.tile([128, 14 * MAXBT], dt.bfloat16, tag="h2")
                for grp in ((0, 2, 4, 6), (1, 3, 5, 7), (8, 10, 12), (9, 11, 13)):
                    banks = _pairs(0, len(grp), bp)
                    pt = {}
                    bank_tiles = []
                    for bank in banks:
                        t = ps.tile(
                            [128, len(bank) * bt], dt.float32,
                            tag="ps", name=f"p2_{ti}_{grp[bank[0]]}",
                        )
                        bank_tiles.append((bank, t))
                        for i, gi in enumerate(bank):
                            pt[grp[gi]] = t[:, i * bt : (i + 1) * bt]
                    blk0 = 0 if grp[0] % 2 == 0 else 3
                    for j in range(3):
                        lead = True
                        for w in grp:
                            f = mm if lead else mm_f
                            f(
                                pt[w][:],
                                w2_sb[:, (blk0 + j) * 128 : (blk0 + j + 1) * 128],
                                h1[:, (w // 2 + j) * bt : (w // 2 + j + 1) * bt],
                                start=(j == 0),
                                stop=(j == 2),
                                skip_group_check=True,
                            )
                            lead = False
                    for bank, t in bank_tiles:
                        c0 = C2[grp[bank[0]]]
                        nc.vector.tensor_relu(
                            h2[:, c0 * bt : (c0 + len(bank)) * bt],
                            t[:, : len(bank) * bt],
                        )

                # conv3: -> relu -> (B,256,8) as two 128-channel tiles,
                # weight-stationary over 4-position blocks.
                h3 = [
                    sb.tile([128, 8 * MAXBT], dt.bfloat16, tag=f"h3_{m}", name=f"h3_{m}")
                    for m in range(2)
                ]
                for m in range(2):
                    for w0 in (0, 4):
                        banks = _pairs(w0, w0 + 4, bp)
                        pt = {}
                        bank_tiles = []
                        for bank in banks:
                            t = ps.tile(
                                [128, len(bank) * bt], dt.float32,
                                tag="ps", name=f"p3_{ti}_{m}_{bank[0]}",
                            )
                            bank_tiles.append((bank, t))
                            for i, w in enumerate(bank):
                                pt[w] = t[:, i * bt : (i + 1) * bt]
                        for k in range(7):
                            lead = True
                            for w in range(w0, w0 + 4):
                                f = mm if lead else mm_f
                                f(
                                    pt[w][:],
                                    w3_sb[:, k * 256 + m * 128 : k * 256 + (m + 1) * 128],
                                    h2[:, C2[w + k] * bt : (C2[w + k] + 1) * bt],
                                    start=(k == 0),
                                    stop=(k == 6),
                                    skip_group_check=True,
                                )
                                lead = False
                        for bank, t in bank_tiles:
                            nc.vector.tensor_relu(
                                h3[m][:, bank[0] * bt : (bank[-1] + 1) * bt],
                                t[:, : len(bank) * bt],
                            )

                # maxpool k=2 s=2: (B,256,8) -> (B,256,4)
                pooled = [
                    sb.tile([128, 4 * MAXBT], dt.bfloat16, tag=f"pool_{m}", name=f"pool_{m}")
                    for m in range(2)
                ]
                for m in range(2):
                    for p in range(4):
                        nc.vector.tensor_max(
                            pooled[m][:, p * bt : (p + 1) * bt],
                            h3[m][:, (2 * p) * bt : (2 * p + 1) * bt],
                            h3[m][:, (2 * p + 1) * bt : (2 * p + 2) * bt],
                        )
                return pooled

            def emit_mlp(ti, pooled):
                bt = TILES[ti]
                boff = tile_offs[ti]

                # mlp1: (B,1024)->(B,256), f = c*4 + wp. For bt=256 both
                # j-halves share one bank and drain with one Relu.
                g1 = sb.tile([128, 2 * MAXBT], dt.bfloat16, tag="g1")
                if False:
                    pmj = ps.tile([128, 2 * bt], dt.float32, tag="ps", name=f"pm1_{ti}")
                    pms = [pmj[:, 0:bt], pmj[:, bt : 2 * bt]]
                else:
                    pms = [
                        ps.tile([128, bt], dt.float32, tag="ps", name=f"pm1_{ti}_{j}")
                        for j in range(2)
                    ]
                for j in range(2):
                    for wp in range(4):
                        for q in range(2):
                            mm(
                                pms[j][:],
                                m1_sb[wp][q][:, j * 128 : (j + 1) * 128],
                                pooled[q][:, wp * bt : (wp + 1) * bt],
                                start=(wp == 0 and q == 0),
                                stop=(wp == 3 and q == 1),
                                skip_group_check=True,
                            )
                if False:
                    nc.vector.tensor_relu(g1[:, 0 : 2 * bt], pmj[:])
                else:
                    for j in range(2):
                        nc.vector.tensor_relu(
                            g1[:, j * bt : (j + 1) * bt], pms[j][:]
                        )

                # mlp2: (B,256)->(B,128)
                g2 = sb.tile([128, MAXBT], dt.bfloat16, tag="g2")
                pm = ps.tile([128, bt], dt.float32, tag="ps", name=f"pm2_{ti}")
                for q in range(2):
                    mm(
                        pm[:], m2_sb[q][:], g1[:, q * bt : (q + 1) * bt],
                        start=(q == 0), stop=(q == 1),
                    )
                nc.vector.tensor_relu(g2[:, :bt], pm[:])

                # mlp3: (B,128)->(B,1)
                pm = ps.tile([1, bt], dt.float32, tag="ps", name=f"pm3_{ti}")
                mm(pm[:], m3_sb[:], g2[:, :bt], start=True, stop=True)
                y_sb = sb.tile([1, MAXBT], dt.float32, tag="y_sb", bufs=2)
                nc.vector.tensor_copy(y_sb[:, :bt], pm[:])
                nc.scalar.dma_start(y_d[:, boff : boff + bt], y_sb[:, :bt])

            pooled_prev = None
            for ti in range(NT):
                h1 = emit_conv1(ti)
                if pooled_prev is not None:
                    emit_mlp(ti - 1, pooled_prev)
                pooled_prev = emit_mid(ti, h1)
            emit_mlp(NT - 1, pooled_prev)

    nc.compile()
    return nc


def _prep_inputs(x, kernel_1, kernel_2, kernel_3, mlp_weight_1, mlp_weight_2, mlp_weight_3):
    """Host-side sharding + layout prep. Returns in_maps (one dict per core)."""
    # conv1 taps duplicated across both 64-column halves so one 128-col
    # LDWEIGHTS serves the (even, odd) pair of M=64 matmuls; the four
    # c-chunks packed side by side for a single DMA.
    k1t = kernel_1.transpose(1, 2, 0).astype(np.float32)  # (512, 3, 64)
    w1 = np.concatenate([k1t, k1t], axis=2).reshape(512, 3 * 128)
    w1 = np.ascontiguousarray(
        w1.reshape(4, 128, 384).transpose(1, 0, 2).reshape(128, 4 * 384)
    ).astype(BF16)
    # conv2 tap-pair blocks for the parity-split h1 layout: column block j is
    # a (128, 128) lhsT whose rows 0-63 multiply h1's even half and rows
    # 64-127 the odd half. Blocks 0-2 serve even output positions
    # ([k0;k1] [k2;k3] [k4;0]), blocks 3-5 odd ones ([0;k0] [k1;k2] [k3;k4]).
    k2t = kernel_2.transpose(1, 2, 0).astype(np.float32)  # (64, 5, 128)
    z = np.zeros((64, 128), np.float32)
    blocks = [
        np.concatenate([k2t[:, 0], k2t[:, 1]], axis=0),
        np.concatenate([k2t[:, 2], k2t[:, 3]], axis=0),
        np.concatenate([k2t[:, 4], z], axis=0),
        np.concatenate([z, k2t[:, 0]], axis=0),
        np.concatenate([k2t[:, 1], k2t[:, 2]], axis=0),
        np.concatenate([k2t[:, 3], k2t[:, 4]], axis=0),
    ]
    w2 = np.concatenate(blocks, axis=1)  # (128, 768)
    w3 = kernel_3.transpose(1, 2, 0).reshape(128, 7 * 256)
    # W1 row f = c*4 + wp  ->  m1 row = wp*256 + c; packed as 8 side-by-side
    # (128, 256) blocks in (wp, q) order.
    m1 = (
        mlp_weight_1.reshape(256, 4, 256)
        .transpose(1, 0, 2)
        .reshape(4, 2, 128, 256)
        .transpose(2, 0, 1, 3)
        .reshape(128, 8 * 256)
    )
    m2 = np.concatenate([mlp_weight_2[0:128], mlp_weight_2[128:256]], axis=1)
    m3 = mlp_weight_3  # (128, 1)
    wb = np.ascontiguousarray(
        np.concatenate([w2, w3, m1, m2, m3], axis=1)
    ).astype(BF16)

    xb = x.astype(BF16)
    in_maps = []
    for c in range(N_CORES):
        # (E, W*BC), tile ti packed at columns [W*off, W*(off+bt)): each
        # (c-chunk, tile) DMA is one contiguous 20*bt*2-byte run/partition.
        xc = np.empty((E, W * BC), dtype=BF16)
        boff = 0
        for bt in TILES:
            blk = xb[c * BC + boff : c * BC + boff + bt]  # (bt, E, W)
            xc[:, W * boff : W * (boff + bt)] = blk.transpose(1, 2, 0).reshape(E, W * bt)
            boff += bt
        in_maps.append({"x": xc, "w1": w1, "wb": wb})
    return in_maps


def run(inputs, trace=False, **kw):
    """Compile (cached), run on 8 cores, return (y_full, BassKernelResults)."""
    from concourse import bass_utils

    if "nc" not in _compiled:
        _compiled["nc"] = _build()
    nc = _compiled["nc"]
    in_maps = _prep_inputs(**inputs)
    res = bass_utils.run_bass_kernel_spmd(
        nc, in_maps, core_ids=list(range(N_CORES)), trace=trace, **kw
    )
    y = np.concatenate(
        [res.results[c]["y"].reshape(BC, 1) for c in range(N_CORES)], axis=0
    )
    return y.astype(np.float32), res


def kernel(**inputs):
    inputs = {k: np.asarray(v) for k, v in inputs.items()}
    y, _ = run(inputs)
    return y


if __name__ == "__main__":
    rng = np.random.default_rng(0)
    inputs = {
        "x": rng.standard_normal((B, E, W), dtype=np.float32),
        "kernel_1": rng.standard_normal((64, 512, 3), dtype=np.float32),
        "kernel_2": rng.standard_normal((128, 64, 5), dtype=np.float32),
        "kernel_3": rng.standard_normal((256, 128, 7), dtype=np.float32),
        "mlp_weight_1": rng.standard_normal((1024, 256), dtype=np.float32),
        "mlp_weight_2": rng.standard_normal((256, 128), dtype=np.float32),
        "mlp_weight_3": rng.standard_normal((128, 1), dtype=np.float32),
    }
    y = kernel(**inputs)
    print("out", y.shape, y.dtype, y[:4, 0])
